# revision 1
# baseline (speedup 1.0000x reference)
"""Multi-head attention (B=2, S=2048, H=16, D=64) on 8 trn2 NeuronCores.

Sharding: the 32 (batch, head) pairs are split 4-per-core (tensor parallel on
heads, data parallel on batch). Each core runs the same Bass program on its
own 4 pairs.

Key host-side trick: the attention mask is per-key and shared by every head
and query, and masked keys contribute exactly 0 to softmax numerator and
denominator. So K and V are compacted to just the unmasked keys per batch
(padded up to a multiple of 256, with padding killed by a -30000 exp bias on
just the padded k-blocks). For the Bernoulli(0.5) mask here that removes
about half of all device work.

Per-pair device algorithm (oriented so softmax needs no cross-partition
reduction and no transposes anywhere):
  1. S^T = K @ Q^T on PE: k on partitions, q on the free axis. Contraction
     dim is D=64, so two k-blocks are packed into the PE array concurrently
     (partition row-tiling at base 0 and 64) with Q^T duplicated into both
     partition halves. The 1/sqrt(D) scale is folded into K on the host.
  2. exp on ScalarE straight out of PSUM (per-partition -30000 bias kills
     padding rows; exp underflows to exactly 0).
  3. ctx = P^T.T @ [V | 1] on PE: the exp'd P^T chunk is the stationary
     operand, V with an appended ones-column streams through, so PSUM
     accumulates the context numerator and the softmax denominator together
     in natural [q, d] layout.
  4. DVE reciprocal of the denominator column + per-partition scalar
     multiply, then chunked DMAs back to HBM.
The ctx/finalize work of each half is interleaved into the next half's
S^T/exp loop so ScalarE (the bottleneck engine) is never starved, and the
per-pair inputs arrive as two large concatenated DMAs (urgent K^T + first
Q^T half in the first one).
"""

from contextlib import ExitStack

import numpy as np
import ml_dtypes

import concourse.bass as bass
import concourse.bacc as bacc
import concourse.tile as tile
from concourse import mybir
from concourse.bass_utils import run_bass_kernel_spmd

N_CORES = 8
B, S, E = 2, 2048, 1024
H, D = 16, 64
PAIRS = B * H // N_CORES        # 4 (b,h) pairs per core
QB = S // 128                   # 16 q-blocks of 128
MASK_BIAS = -30000.0

f32 = mybir.dt.float32
bf16 = mybir.dt.bfloat16
BF16 = ml_dtypes.bfloat16


def _emit_ctx(nc, cxp, small, ptt, vot, ot, qb_list, kb, tag="cx"):
    """ctx = P^T.T @ [V|1] for the given q-blocks, then normalize into ot."""
    for qb in qb_list:
        cx = cxp.tile([128, D + 1], f32, tag=tag)
        for c in range(kb):
            nc.tensor.matmul(
                cx[:],
                lhsT=ptt[:, c, qb * 128:(qb + 1) * 128],
                rhs=vot[:, c, :],
                start=(c == 0), stop=(c == kb - 1),
            )
        rec = small.tile([128, 1], f32, tag="rec")
        nc.vector.reciprocal(out=rec[:], in_=cx[:, D:D + 1])
        nc.vector.tensor_scalar_mul(
            out=ot[:, qb * D:(qb + 1) * D],
            in0=cx[:, 0:D],
            scalar1=rec[:],
        )


def _attn_tile(ctx, tc, inA, inB, out, kb):
    nc = tc.nc
    Exp = mybir.ActivationFunctionType.Exp
    step_blocks = [(2 * i, 2 * i + 1) for i in range(kb // 2)]
    if kb % 2:
        step_blocks.append((kb - 1, None))
    steps = len(step_blocks)
    WK = steps * 128                     # packed K^T width
    NQC = S // 512                       # q-chunks of 512

    io = ctx.enter_context(tc.tile_pool(name="io", bufs=2))
    ptp = ctx.enter_context(tc.tile_pool(name="pt", bufs=2))
    outp = ctx.enter_context(tc.tile_pool(name="outp", bufs=2))
    small = ctx.enter_context(tc.tile_pool(name="small", bufs=4))
    scp = ctx.enter_context(tc.tile_pool(name="scores", bufs=2, space="PSUM"))
    cxp = ctx.enter_context(tc.tile_pool(name="ctx", bufs=2, space="PSUM"))

    warm = small.tile([128, 1], f32, tag="warm")
    nc.vector.memset(warm[:], 0.0)
    nc.scalar.activation(warm[:], warm[:], Exp, bias=0.0, scale=1.0)
    wsrc = small.tile([128, 512], bf16, tag="wsrc")
    nc.vector.memset(wsrc[:], 0.0)
    for _ in range(2):
        wps = scp.tile([128, 512], f32, tag="sc")
        nc.tensor.matmul(wps[:], lhsT=wsrc[:, 0:128], rhs=wsrc[:],
                         start=True, stop=True)

    def flush_ctx(pend, qbs, dma_chunk=QB // 2, pool=cxp, tag="cx"):
        pptt, pvot, pot, pout, st = pend
        for qb in qbs:
            cx = pool.tile([128, D + 1], f32, tag=tag)
            for c in range(kb):
                nc.tensor.matmul(
                    cx[:],
                    lhsT=pptt[:, qb // 4, c, (qb % 4) * 128:(qb % 4) * 128 + 128],
                    rhs=pvot[:, c, :],
                    start=(c == 0), stop=(c == kb - 1),
                )
            rec = small.tile([128, 1], f32, tag="rec")
            nc.vector.reciprocal(out=rec[:], in_=cx[:, D:D + 1])
            nc.vector.tensor_scalar_mul(
                out=pot[:, qb * D:(qb + 1) * D], in0=cx[:, 0:D], scalar1=rec[:])
        st[0] += len(qbs)
        out_v = pout.rearrange("(qb q) d -> q qb d", qb=QB)
        ot_v = pot.rearrange("q (qb d) -> q qb d", qb=QB)
        while st[0] - st[1] >= dma_chunk or (st[0] == QB and st[1] < QB):
            lo = st[1]
            hi = min(lo + dma_chunk, st[0])
            nc.sync.dma_start(out=out_v[:, lo:hi], in_=ot_v[:, lo:hi])
            st[1] = hi

    pending = None
    pend_qbs = []
    for p in range(PAIRS):
        iA = io.tile([128, WK + 1024], bf16, tag="iA")
        nc.sync.dma_start(out=iA[:, 0:WK + 512], in_=inA[p][:, 0:WK + 512])
        nc.sync.dma_start(out=iA[:, WK + 512:], in_=inA[p][:, WK + 512:])
        iB = io.tile([128, 1024 + kb * (D + 1)], bf16, tag="iB")
        nc.sync.dma_start(out=iB[:], in_=inB[p])
        kTt = iA[:, 0:WK]
        vot = iB[:, 1024:].rearrange("q (c d) -> q c d", c=kb)

        # P^T laid out [128, q-chunk, block, 512]: consecutive blocks of one
        # q-chunk are contiguous, so one exp op spans a whole 3-block slot
        ptt = ptp.tile([128, NQC, kb, 512], bf16, tag="pt")
        ot = outp.tile([128, QB * D], f32, tag="out")
        st = [0, 0]
        for qc in range(NQC):            # q-chunks of 512
            qsrc = (iA[:, WK:WK + 1024] if qc < 2 else iB[:, 0:1024])
            q0 = (qc % 2) * 512
            tiles = {}                   # tile index -> (tile, n_filled)
            for s, (blkA, blkB) in enumerate(step_blocks):
                for blk in (blkA, blkB):
                    if blk is None:
                        continue
                    t = blk // 3
                    if t not in tiles:
                        width = min(3, kb - 3 * t) * 512
                        sct = scp.tile([128, width], f32, tag="sc")
                        tiles[t] = [sct, 0]
                    sct, _n = tiles[t]
                    pos = (blk % 3) * 512
                    base = 0 if blk == blkA else 64
                    nc.tensor.matmul(
                        sct[:, pos:pos + 512],
                        lhsT=kTt[base:base + 64, s * 128:(s + 1) * 128],
                        rhs=qsrc[base:base + 64, q0:q0 + 512],
                        start=True, stop=True,
                    )
                    tiles[t][1] += 1
                    full = min(3, kb - 3 * t)
                    if tiles[t][1] == full:
                        nc.scalar.activation(
                            ptt[:, qc, 3 * t:3 * t + full, :].rearrange(
                                "q a b -> q (a b)"),
                            sct[:], Exp, bias=0.0, scale=1.0)
                if pending is not None and pend_qbs:
                    take = -(-len(pend_qbs) // (steps - s))
                    flush_ctx(pending, pend_qbs[:take])
                    pend_qbs = pend_qbs[take:]
            if pending is not None and pend_qbs:
                flush_ctx(pending, pend_qbs)
            pending = (ptt, vot, ot, out[p], st)
            pend_qbs = list(range(qc * 4, qc * 4 + 4))

    for i, qb in enumerate(pend_qbs):
        pool, tag = (scp, "sc") if i % 2 else (cxp, "cx")
        flush_ctx(pending, [qb], dma_chunk=2, pool=pool, tag=tag)


def _build(kb, pad_block):
    """Compile the SPMD program for kb k-blocks (kb*128 key capacity),
    where k-blocks >= pad_block may contain padded keys."""
    nc = bacc.Bacc("TRN2", target_bir_lowering=False, debug=False,
                   num_devices=N_CORES)
    WK = ((kb + 1) // 2) * 128
    inA = nc.dram_tensor("inA", [PAIRS, 128, WK + 1024], bf16,
                         kind="ExternalInput").ap()
    inB = nc.dram_tensor("inB", [PAIRS, 128, 1024 + kb * (D + 1)], bf16,
                         kind="ExternalInput").ap()
    out = nc.dram_tensor("out", [PAIRS, S, D], f32, kind="ExternalOutput").ap()
    with tile.TileContext(nc) as tc, ExitStack() as es:
        _attn_tile(es, tc, inA, inB, out, kb)
    nc.compile()
    return nc


_NC_CACHE = {}


def _get_nc(kb, pad_block):
    key = (kb, pad_block)
    if key not in _NC_CACHE:
        _NC_CACHE[key] = _build(kb, pad_block)
    return _NC_CACHE[key]


def _prep_inputs(query, key, value, attention_mask):
    q = np.asarray(query, np.float32)
    k = np.asarray(key, np.float32)
    v = np.asarray(value, np.float32)
    m = np.asarray(attention_mask).reshape(B, S)

    # --- compact K/V to unmasked keys (shared by all heads of a batch) ---
    counts = (m != 0).sum(axis=1)
    cap = max(128, int(-(-int(counts.max()) // 128)) * 128)
    cap = min(cap, S)
    kb = cap // 128
    pad_block = int(counts.min()) // 128
    kc = np.zeros((B, cap, E), np.float32)
    vc = np.zeros((B, cap, E), np.float32)
    bias_c = np.full((B, cap), np.float32(MASK_BIAS), np.float32)
    for b in range(B):
        idx = np.nonzero(m[b])[0]
        n = len(idx)
        kc[b, :n] = k[b, idx]
        vc[b, :n] = v[b, idx]
        bias_c[b, :n] = 0.0

    # [B, S, E] -> per-(b,h) transposed heads
    qT = q.reshape(B, S, H, D).transpose(0, 2, 3, 1).reshape(B * H, D, S)
    kT = (kc * (D ** -0.5)).reshape(B, cap, H, D).transpose(0, 2, 3, 1)
    kT = kT.reshape(B * H, D, cap)

    # Q^T duplicated into both partition halves for PE row-packing
    qT_dup = np.concatenate([qT, qT], axis=1).astype(BF16)      # [32, 128, S]

    # K^T packed: partitions 0:64 = even k-blocks, 64:128 = odd k-blocks
    # (zero phantom block appended when kb is odd)
    kbe = kb + (kb % 2)
    kTp = np.zeros((B * H, D, kbe * 128), np.float32)
    kTp[:, :, :cap] = kT
    kT_pack = (kTp.reshape(B * H, D, kbe // 2, 2, 128)
                  .transpose(0, 3, 1, 2, 4)
                  .reshape(B * H, 128, (kbe // 2) * 128).astype(BF16))

    # V chunks with appended ones column: [32, 128, kb, 65]
    v_r = vc.reshape(B, kb, 128, H, D).transpose(0, 3, 2, 1, 4)
    vo = np.zeros((B, H, 128, kb, D + 1), np.float32)
    vo[..., :D] = v_r
    # denominator ones-column: 0 for padded keys kills them without any bias
    kidx = np.arange(cap).reshape(kb, 128)
    for b in range(B):
        n = int((m[b] != 0).sum())
        vo[b, :, :, :, D] = (kidx.T[None] < n)
    vo = vo.reshape(B * H, 128, kb * (D + 1)).astype(BF16)

    # bias laid out [kk, kblk], broadcast over heads
    mb = bias_c.reshape(B, kb, 128).transpose(0, 2, 1)          # [B, 128, kb]
    mb = np.broadcast_to(mb[:, None], (B, H, 128, kb)).reshape(B * H, 128, kb)
    mb = np.ascontiguousarray(mb, np.float32)

    inA = np.concatenate([kT_pack, qT_dup[:, :, 0:1024]], axis=2)
    inB = np.concatenate([qT_dup[:, :, 1024:S], vo], axis=2)

    in_maps = []
    for c in range(N_CORES):
        sl = slice(c * PAIRS, (c + 1) * PAIRS)
        in_maps.append({
            "inA": np.ascontiguousarray(inA[sl]),
            "inB": np.ascontiguousarray(inB[sl]),
        })
    return in_maps, kb, pad_block


def kernel(query, key, value, attention_mask, **run_kwargs):
    in_maps, kb, pad_block = _prep_inputs(query, key, value, attention_mask)
    nc = _get_nc(kb, pad_block)
    res = run_bass_kernel_spmd(nc, in_maps, core_ids=list(range(N_CORES)),
                               **run_kwargs)
    outs = np.stack([r["out"] for r in res.results])            # [8, 4, S, D]
    full = outs.reshape(B, H, S, D).transpose(0, 2, 1, 3).reshape(B, S, E)
    kernel.last_results = res
    return np.ascontiguousarray(full, np.float32)

